# revision 33
# baseline (speedup 1.0000x reference)
"""CP tensor-regression-layer kernel for Trainium2 (8 NeuronCores).

Computation (matches the reference einsum pair):
    t[b, r]  = sum_{i,j,k} x[b,i,j,k] * f0[i,r] * f1[j,r] * f2[k,r]
    out[b,c] = sum_r t[b,r] * weight[r] * f3[c,r] + bias[0]

Strategy: data-parallel over the batch dim (32 batches per core, CP
factors replicated).  Per core the big contraction is restructured as
    z[r, b, k] = sum_{ij} (f0[i,r]*f1[j,r]) * x[b, ij, k]
a K=2304 matmul against the Khatri-Rao product of f0 and f1, run as
18 K-chunks of 128 partitions.  Everything the PE touches is fp16
(pre-cast on the host; ~1e-3 relative error at half the byte cost),
so the HBM x-stream is ~7.1 MB/core — the roofline term, reached by
streaming big paired-chunk transfers on both HWDGE rings (few
completion-sem lanes exist, so small DMA counts keep the rings full).
Single-chunk transfers bracket the stream so the first matmul isn't
gated behind a double-size transfer and the last chunk lands as early
as possible.

The chunks feed two [64, 1536] PSUM accumulators (chunks 0-9 / rest).
The k-contraction against f2*weight is a full-width DVE
multiply+reduce per accumulator — the first runs mid-stream, only the
second is in the tail.  The two rank-partials merge for free inside
the class projection by accumulating two PE matmuls into the same
PSUM banks.  The bias-add/copy out of PSUM is split between the ACT
and DVE engines, and the two output halves leave on different HWDGE
rings.  The factor matrices arrive pre-gathered into the
[128-partition, chunk, rank] layout (host does indexed replication
only — the Khatri-Rao product itself is a DVE multiply on device) and
lead their rings so the PE can start as soon as chunk 0 lands.
"""

import os

import numpy as np

_B, _M1, _M2, _M3, _C, _R = 256, 48, 48, 48, 1000, 64
_NCORES = 8
_BL = _B // _NCORES          # 32 batches per core
_IJ = _M1 * _M2              # 2304 contraction size (i,j fused)
_NCH = _IJ // 128            # 18 K-chunks of 128 partitions
_KB = _BL * _M3              # 1536 moving columns (b,k fused)
_SL = 512                    # matmul slice width (one PSUM bank, fp32)
_NA = 10                     # chunks 0-9 -> accumulator A, rest -> B
_NCST = _M3 + 2              # packed consts: f2t|w|bias
_CH = _C // 2

_cache = {}


def _split_excess_waits(nc, mybir, max_waits=1):
    """Walrus in this container rejects >1 sync-wait per instruction
    ("Too many sync wait commands").  Move excess waits onto chained
    NoOps inserted just before the offending instruction (same engine,
    so program order preserves the gating)."""
    for bb in nc.m.functions[0].blocks:
        insts = bb.instructions
        i = 0
        while i < len(insts):
            inst = insts[i]
            si = getattr(inst, "sync_info", None)
            waits = list(si.on_wait) if si is not None and si.on_wait else []
            if len(waits) > max_waits:
                rest, keep = waits[:-max_waits], waits[-max_waits:]
                pos = i
                for j in range(0, len(rest), max_waits):
                    nop = mybir.InstNoOp(
                        name=f"I-waitsplit-{nc.next_id()}",
                        engine=inst.engine,
                        ins=[],
                        outs=[],
                        sync_info=mybir.SyncInfo(
                            on_wait=list(rest[j : j + max_waits]), on_update=[]
                        ),
                    )
                    nc.register_instruction(nop)
                    insts.insert(pos, nop)
                    pos += 1
                    i += 1
                si.on_wait = keep
            i += 1


def _bcast(ap, bass, shape3):
    """AP broadcast helper: make a 3D view with a stride-0 middle dim."""
    try:
        return ap.unsqueeze(1).broadcast_to(shape3)
    except Exception:
        a = ap.ap
        return bass.AP(
            tensor=ap.tensor,
            offset=ap.offset,
            ap=[list(a[0]), [0, shape3[1]], list(a[1])],
        )


def _build_program():
    import concourse.bass as bass
    import concourse.tile as tile
    from concourse import mybir

    f32 = mybir.dt.float32
    f16 = mybir.dt.float16

    nc = bass.Bass("TRN2", target_bir_lowering=False, debug=False,
                   num_devices=_NCORES)

    x_d = nc.dram_tensor("x", [128, _NCH, _BL, _M3], f16, kind="ExternalInput")
    f0r_d = nc.dram_tensor("f0r", [128, _NCH, _R], f16, kind="ExternalInput")
    f1r_d = nc.dram_tensor("f1r", [128, _NCH, _R], f16, kind="ExternalInput")
    cst_d = nc.dram_tensor("cst", [_R, _NCST], f32, kind="ExternalInput")
    f3t_d = nc.dram_tensor("f3t", [_R, _C], f16, kind="ExternalInput")
    out_d = nc.dram_tensor("out", [_BL, _C], f16, kind="ExternalOutput")

    with tile.TileContext(nc) as tc:
        with (
            tc.tile_pool(name="consts", bufs=1) as consts,
            tc.tile_pool(name="xp", bufs=1) as xp,
            tc.tile_pool(name="work", bufs=1) as work,
            tc.tile_pool(name="pz", bufs=1, space=bass.MemorySpace.PSUM) as pz,
        ):
            xs = {}       # chunk -> (tile, sub-index)
            f0r = consts.tile([128, _NCH, _R], f16)
            f1r = consts.tile([128, _NCH, _R], f16)
            f3t = consts.tile([_R, _C], f16)

            def x_dma(eng, m0, n):
                # partition-major x layout measures faster than
                # chunk-major (strided 3KB reads suit the 16-way SDMA
                # engine striping better than one contiguous region)
                xt = xp.tile([128, n, _BL, _M3], f16, tag=f"x{m0}")
                eng.dma_start(out=xt[:], in_=x_d[:, m0 : m0 + n])
                for u in range(n):
                    xs[m0 + u] = (xt, u)

            # kr factors lead their rings; single-chunk transfers
            # bracket the stream (fast first matmul, early last chunk);
            # pair transfers in between keep the total DMA count at 13
            # — only ~8 HWDGE completion-sem lanes exist, and many more
            # DMAs than that throttles the late issues, starving the
            # stream tail.
            nc.sync.dma_start(out=f0r[:], in_=f0r_d[:])
            nc.scalar.dma_start(out=f1r[:], in_=f1r_d[:])
            x_dma(nc.sync, 0, 1)
            x_dma(nc.scalar, 1, 1)
            x_dma(nc.sync, 2, 2)
            x_dma(nc.scalar, 4, 2)
            x_dma(nc.sync, 6, 2)
            x_dma(nc.scalar, 8, 2)
            x_dma(nc.sync, 10, 2)
            x_dma(nc.scalar, 12, 2)
            x_dma(nc.sync, 14, 2)
            x_dma(nc.scalar, 16, 1)
            x_dma(nc.scalar, 17, 1)

            # small consts + the tail-only f3t on the gpsimd (SWDGE)
            # queue, keeping the HWDGE rings pure x/factor stream
            cst = consts.tile([_R, _NCST], f32)
            nc.gpsimd.dma_start(out=cst[:], in_=cst_d[:])
            nc.gpsimd.dma_start(out=f3t[:], in_=f3t_d[:])
            f2t = cst[:, 0:_M3]
            wsb = cst[:, _NCST - 2 : _NCST - 1]
            bsb = cst[:_BL, _NCST - 1 : _NCST]

            # touch the ACT Identity table now so the tail bias-add
            # doesn't pay the on-demand ACT_TABLE_LOAD (~1.3us)
            warm = consts.tile([1, 1], f32)
            nc.scalar.add(warm[:], cst[:1, _NCST - 2 : _NCST - 1], 0.0)

            # ---- KR = f0 (x) f1 in the [p, m, r] layout the PE
            # consumes: fp16 DVE multiplies on the pre-gathered factors,
            # first chunk split out so matmul 0 starts the moment its
            # data lands ----
            kr = consts.tile([128, _NCH, _R], f16)
            with nc.allow_low_precision(reason="fp16 weights for PE"):
                for g0, g1 in ((0, 2), (2, _NA), (_NA, _NCH)):
                    nc.vector.tensor_mul(
                        kr[:, g0:g1], f0r[:, g0:g1], f1r[:, g0:g1]
                    )

            # weight folds into f2 (needed first by the mid-stream
            # contraction of accumulator A)
            f2tw = consts.tile([_R, _M3], f32)
            nc.vector.tensor_scalar_mul(f2tw[:], f2t, wsb)

            # ---- main contraction into two [64, 1536] accumulators ----
            za = pz.tile([_R, _KB], f32, tag="za")
            zb = pz.tile([_R, _KB], f32, tag="zb")

            def emit_chunk(m, ztile, start, stop):
                xt, u = xs[m]
                xm_f = xt[:, u].rearrange("p b k -> p (b k)")
                for s in range(_KB // _SL):
                    nc.tensor.matmul(
                        ztile[:, s * _SL : (s + 1) * _SL],
                        lhsT=kr[:, m, :],
                        rhs=xm_f[:, s * _SL : (s + 1) * _SL],
                        start=start,
                        stop=stop,
                    )

            def k_contract(ztile, zftag, ttag):
                """Full-width multiply + reduce of one accumulator."""
                zf = work.tile([_R, _BL, _M3], f16, tag=zftag)
                t_ = work.tile([_R, _BL], f16, tag=ttag)
                z3 = ztile[:].rearrange("r (b k) -> r b k", k=_M3)
                with nc.allow_low_precision(reason="fp16 k-reduce"):
                    nc.vector.tensor_mul(
                        zf[:], z3, _bcast(f2tw[:], bass, (_R, _BL, _M3))
                    )
                    nc.vector.reduce_sum(
                        t_[:], zf[:], axis=mybir.AxisListType.X
                    )
                return t_

            for m in range(_NA):
                emit_chunk(m, za, m == 0, m == _NA - 1)
            ta = k_contract(za, "zfa", "ta")       # overlaps chunks 12-17
            for m in range(_NA, _NCH):
                emit_chunk(m, zb, m == _NA, m == _NCH - 1)

            osb = work.tile([_BL, _C], f16, tag="osb")
            with tc.tile_pool(
                name="po", bufs=1, space=bass.MemorySpace.PSUM
            ) as po:
                op0 = po.tile([_BL, _CH], f32, tag="op0")
                op1 = po.tile([_BL, _CH], f32, tag="op1")
                ops = [op0, op1]
                # projection of the A-half: on the PE right after the
                # last chunk, overlapping the B-half's DVE contraction
                for s in (0, 1):
                    nc.tensor.matmul(
                        ops[s][:], lhsT=ta[:],
                        rhs=f3t[:, s * _CH : (s + 1) * _CH],
                        start=True, stop=False,
                    )
                tb = k_contract(zb, "zfb", "tb")   # the only tail contract
                for s in (0, 1):
                    nc.tensor.matmul(
                        ops[s][:], lhsT=tb[:],
                        rhs=f3t[:, s * _CH : (s + 1) * _CH],
                        start=False, stop=True,
                    )
                # bias-add + PSUM->SBUF copy split across ACT and DVE,
                # each output half leaving on its own HWDGE ring
                with nc.allow_low_precision(reason="fp16 output"):
                    nc.scalar.add(osb[:, 0:_CH], ops[0][:], bsb)
                    nc.sync.dma_start(
                        out=out_d[:, 0:_CH], in_=osb[:, 0:_CH],
                        single_packet=True,
                    )
                    nc.vector.tensor_scalar_add(
                        osb[:, _CH:_C], ops[1][:], bsb
                    )
                    nc.scalar.dma_start(
                        out=out_d[:, _CH:_C], in_=osb[:, _CH:_C],
                        single_packet=True,
                    )

    _split_excess_waits(nc, mybir)
    return nc


def _get_program():
    if "nc" not in _cache:
        _cache["nc"] = _build_program()
    return _cache["nc"]


def _host_prep(x, weight, f0, f1, f2, f3, bias):
    """Layout/precision prep only: shard x over cores (batch dim) in a
    DMA-friendly fp16 layout, replicate factor rows into the
    [partition, chunk, rank] gather layout, pack the small constants
    into one tensor."""
    x16 = np.asarray(x, dtype=np.float32).astype(np.float16)
    ij = np.arange(_IJ)
    f0_16 = np.asarray(f0, np.float32).astype(np.float16)
    f1_16 = np.asarray(f1, np.float32).astype(np.float16)
    # [ij, r] -> [p, m, r] with ij = 128*m + p
    f0r = np.ascontiguousarray(
        f0_16[ij // _M2].reshape(_NCH, 128, _R).transpose(1, 0, 2)
    )
    f1r = np.ascontiguousarray(
        f1_16[ij % _M2].reshape(_NCH, 128, _R).transpose(1, 0, 2)
    )
    cst = np.empty((_R, _NCST), np.float32)
    cst[:, 0:_M3] = np.asarray(f2, np.float32).T
    cst[:, _NCST - 2] = np.asarray(weight, np.float32)
    cst[:, _NCST - 1] = np.float32(np.asarray(bias, np.float32)[0])
    f3t = np.ascontiguousarray(
        np.asarray(f3, np.float32).T.astype(np.float16)
    )
    in_maps = []
    for c in range(_NCORES):
        xc = x16[c * _BL : (c + 1) * _BL]
        # [b, ij, k] -> [p, m, b, k] with ij = 128*m + p
        xd = np.ascontiguousarray(
            xc.reshape(_BL, _NCH, 128, _M3).transpose(2, 1, 0, 3)
        )
        in_maps.append(
            {"x": xd, "f0r": f0r, "f1r": f1r, "cst": cst, "f3t": f3t}
        )
    return in_maps


LAST_EXEC_NS = None


def kernel(x, weight, f0, f1, f2, f3, bias):
    global LAST_EXEC_NS
    from concourse.bass_utils import run_bass_kernel_spmd

    nc = _get_program()
    in_maps = _host_prep(x, weight, f0, f1, f2, f3, bias)
    trace = bool(int(os.environ.get("BASS_KERNEL_TRACE", "0")))
    res = run_bass_kernel_spmd(nc, in_maps, list(range(_NCORES)), trace=trace)
    LAST_EXEC_NS = res.exec_time_ns
    out = np.concatenate([res.results[c]["out"] for c in range(_NCORES)], axis=0)
    return np.ascontiguousarray(out.astype(np.float32, copy=False))
